# revision 44
# baseline (speedup 1.0000x reference)
"""Trainium2 Bass kernel for grouped-query attention with qk-norm.

Problem (hardcoded): x(2,2048,1024) @ Wq(1024,1024) / Wkv(1024,512),
16 query heads, 4 kv heads, head_dim 64, k_scale(16,1,64) applied to the
group-broadcast k. Output (2,2048,1024).

Sharding: 8 cores = batch(2) x kv_heads(4). Each core computes its batch's
4 query heads against its kv head over the full 2048x2048 score matrix.

Device kernel layout choices:
- Host passes x transposed (xT: dim on partitions) so all projection
  matmuls contract over dim with no on-device transposes.
- k_scale is folded into Wk host-side: (x@Wk)*ks == x@(Wk*diag(ks)),
  giving a per-query-head scaled kkT directly from the projection.
- Scores are computed transposed (S^T: keys on partitions, queries free)
  so that exp(S^T) tiles feed the PV matmul directly as the moving
  operand (no P transpose).
- Softmax skips the max-subtraction (inputs are bounded; exp stays well
  inside fp32 range) and normalizes after PV via an appended ones-row in
  the V stationary operand (row 64 of the PV psum accumulates sum(exp)).
- Output is returned transposed per head (oT: 4*64 x 2048); the host
  transposes during the gather.
- Matmul inputs are float32r (fp32 storage, reduced-precision multiply,
  4x the fp32 throughput at moving-dim >= 256).
"""

import os
from contextlib import ExitStack

import numpy as np

import concourse.bacc as bacc
import concourse.mybir as mybir
import concourse.tile as tile
from concourse.bass_utils import run_bass_kernel_spmd

# Problem constants
B, N, DIM = 2, 2048, 1024
HEADS, KV_HEADS, DH = 16, 4, 64
G = HEADS // KV_HEADS  # query heads per kv head (4)
NCORES = 8
P = 128
KT = DIM // P  # 8 contraction tiles over dim
IC = 512  # query-chunk width
NI = N // IC  # 4
NJ = N // P  # 16 key tiles
SCALE = DH**-0.5

F32 = mybir.dt.float32
F16 = mybir.dt.float16
DMM = F16  # all matmul operands fp16: 1 row/cycle, fast stationary loads


def emit_kernel(ctx, tc, xT, wq, wkv, dupeye, eyef, oT):
    nc = tc.nc
    Exp = mybir.ActivationFunctionType.Exp
    mult = mybir.AluOpType.mult

    wpool = ctx.enter_context(tc.tile_pool(name="w", bufs=1))
    qkpool = ctx.enter_context(tc.tile_pool(name="qk", bufs=1))
    ptpool = ctx.enter_context(tc.tile_pool(name="pt", bufs=6))
    npool = ctx.enter_context(tc.tile_pool(name="norm", bufs=2))

    # --- persistent SBUF tensors ---
    ones_sb = wpool.tile([P, DH], DMM, tag="ones")  # warmup matmul operand
    # f32 identity at partitions 64-127 for the vT transpose (v lives there)
    eye_sb = wpool.tile([P, DH], F32, tag="eye")
    # [I | I] stationary that duplicates k rows 0-63 into psum rows 64-127
    dup_sb = wpool.tile([DH, P], DMM, tag="dup")
    qT = [qkpool.tile([P, N], DMM, name=f"qT{hp}", tag=f"qT{hp}") for hp in range(2)]
    # single unscaled k (k_scale folded into Wq host-side), duplicated on
    # partitions 0-63 and 64-127 to serve both QK tile positions
    kkT = qkpool.tile([P, N], DMM, tag="kkT")
    vaug = qkpool.tile([P, NJ * (DH + 1)], F16, tag="vaug")
    nc.any.memset(vaug[:], 1.0)
    nc.any.memset(ones_sb[:], 1.0)
    warm = qkpool.tile([1, 1], F32, tag="warm")
    nc.any.memset(warm[:], 1.0)
    nc.scalar.activation(warm[:], warm[:], Exp)
    nc.sync.dma_start(dup_sb[:], dupeye[:, :])
    nc.sync.dma_start(eye_sb[DH:P, :], eyef[:, :])

    # unnormalized output + softmax sums; the host divides during gather
    sums_d = nc.dram_tensor("sums_d", (G, N), F32, kind="ExternalOutput").ap()
    o_acc = [
        npool.tile([DH + 1, N], F32, name=f"oacc{h}", tag=f"oacc{h}", bufs=1)
        for h in range(G)
    ]

    def qk_exp(hp, ic, jt, pt):
        csl = slice(ic * IC, (ic + 1) * IC)
        st = apsum.tile([P, 2 * IC], F32, tag="s", bufs=2, name="st")
        for half in range(2):
            rsl = slice(half * 64, half * 64 + 64)
            nc.tensor.matmul(
                st[:, half * IC : (half + 1) * IC],
                kkT[rsl, jt * P : (jt + 1) * P],
                qT[hp][rsl, csl],
                start=True,
                stop=True,
                tile_position=(half * 64, 0),
            )
        nc.scalar.activation(pt[:], st[:], Exp, scale=SCALE)

    def pv_mm(o_ps, jt, pt):
        for half in range(2):
            nc.tensor.matmul(
                o_ps[half][:],
                vaug[:, jt * (DH + 1) : (jt + 1) * (DH + 1)],
                pt[:, half * IC : (half + 1) * IC],
                start=(jt == 0),
                stop=(jt == NJ - 1),
            )

    def drain_block(hp, ic, o_ps):
        for half in range(2):
            h = 2 * hp + half
            csl = slice(ic * IC, (ic + 1) * IC)
            nc.vector.tensor_copy(o_acc[h][:, csl], o_ps[half][:])
            nc.sync.dma_start(
                sums_d[h : h + 1, csl], o_acc[h][DH : DH + 1, csl]
            )
            nc.sync.dma_start(oT[h * DH : (h + 1) * DH, csl], o_acc[h][0:DH, csl])

    # S-tile pool lives for the whole kernel so early attention blocks can
    # overlap the projection phase (PV is deferred; its accumulator banks
    # open only after the projection psum pool closes).
    apsum = ctx.enter_context(tc.tile_pool(name="ap", bufs=2, space="PSUM"))
    opool = ctx.enter_context(tc.tile_pool(name="op", bufs=1, space="PSUM"))
    # Dummy matmuls during the initial DMA wait keep the PE HAM activity
    # monitor busy so real projections start at 2.4GHz instead of 1.2.
    for _ in range(28):
        wt = apsum.tile([DH, IC], F32, tag="s", name="wt", bufs=2)
        nc.tensor.matmul(
            wt[:, 0:DH], ones_sb[:, 0:DH], ones_sb[:, 0:DH],
            start=True, stop=True,
        )

    # --- projections (fp16 inputs): qT (d on partitions) + packed [k|v] ---
    with tc.tile_pool(name="xw", bufs=1) as xwpool:
        wq_sb = xwpool.tile([P, KT * 256], F16, tag="wq")
        wkv_sb = xwpool.tile([P, KT * P], F16, tag="wkv")
        xts = xwpool.tile([P, KT * N], F16, tag="xt")  # 4MB
        # v occupies partitions 64-127 (its home in the packed kv psum)
        vT_sb = xwpool.tile([P, N], F32, tag="vT")

        qs = [nc.gpsimd, nc.scalar, nc.sync]

        def dma_x(kt, ic, eng):
            r = slice(kt * P, (kt + 1) * P)
            csl = slice(ic * IC, (ic + 1) * IC)
            eng.dma_start(
                xts[:, kt * N + ic * IC : kt * N + (ic + 1) * IC], xT[r, csl]
            )

        # weights first, spread over all three DMA queues (gpsimd SW-DGE +
        # scalar/sync HW-DGE), then x round-robin on the same queues
        n = 0
        for kt in range(KT):
            r = slice(kt * P, (kt + 1) * P)
            qs[n % 3].dma_start(wq_sb[:, kt * 256 : (kt + 1) * 256], wq[r, :])
            qs[(n + 1) % 3].dma_start(wkv_sb[:, kt * P : (kt + 1) * P], wkv[r, :])
            n += 2
        for ic in range(NI):
            for kt in range(KT):
                dma_x(kt, ic, qs[n % 3])
                n += 1

        def proj_wave(ic, pp):
            # one wave = every projection chain that consumes xts chunk ic
            csl = slice(ic * IC, (ic + 1) * IC)
            for hp in range(2):
                ps = pp.tile([P, IC], F32, tag="pj", name="pjt", bufs=2)
                for kt in range(KT):
                    c0 = kt * 256 + hp * 128
                    nc.tensor.matmul(
                        ps[:],
                        wq_sb[:, c0 : c0 + 128],
                        xts[:, kt * N + ic * IC : kt * N + (ic + 1) * IC],
                        start=(kt == 0),
                        stop=(kt == KT - 1),
                    )
                nc.vector.tensor_copy(qT[hp][:, csl], ps[:])
            # packed [k | v] projection: k on rows 0-63, v on rows 64-127
            ps = pp.tile([P, IC], F32, tag="pj", name="pjkv", bufs=2)
            for kt in range(KT):
                nc.tensor.matmul(
                    ps[:],
                    wkv_sb[:, kt * P : (kt + 1) * P],
                    xts[:, kt * N + ic * IC : kt * N + (ic + 1) * IC],
                    start=(kt == 0),
                    stop=(kt == KT - 1),
                )
            nc.vector.tensor_copy(kkT[0:DH, csl], ps[0:DH, :])
            nc.vector.tensor_copy(vT_sb[DH:P, csl], ps[DH:P, :])
            # duplicate k into kkT rows 64-127 via the [I|I] stationary
            psd = pp.tile([P, IC], F32, tag="pj", name="pjd", bufs=2)
            nc.tensor.matmul(
                psd[:], dup_sb[:], kkT[0:DH, csl], start=True, stop=True
            )
            nc.vector.tensor_copy(kkT[DH:P, csl], psd[DH:P, :])
            for jt in range(4 * ic, 4 * ic + 4):
                pv = pp.tile([P, DH], F32, tag="pj", bufs=2, name="pvt")
                nc.tensor.transpose(
                    pv[:], vT_sb[DH:P, jt * P : (jt + 1) * P], eye_sb[DH:P, :]
                )
                nc.vector.tensor_copy(
                    vaug[:, jt * (DH + 1) : jt * (DH + 1) + DH], pv[:]
                )

        # Block (0,0) runs its full QK+exp+PV chain live inside the
        # projection waves (st bufs=2 frees the accumulator banks for it).
        # Block (1,0) runs QK+exp during the waves into held tiles; its PVs
        # are woven through blocks (0,1..3) so ACT never sees a PV burst.
        pt_hold = [
            ptpool.tile([P, 2 * IC], F16, name=f"pth{j}", tag=f"pth{j}", bufs=1)
            for j in range(NJ)
        ]
        o_ps00 = [
            opool.tile([DH + 1, IC], F32, name=f"o00_{i}", tag=f"ops{i}", bufs=1)
            for i in range(2)
        ]
        with tc.tile_pool(name="pp", bufs=2, space="PSUM") as pp:
            pend = None
            for w in range(NI):
                proj_wave(w, pp)
                for jt in range(4 * w, 4 * w + 4):
                    pt = ptpool.tile([P, 2 * IC], F16, tag="pt")
                    qk_exp(0, 0, jt, pt)
                    qk_exp(1, 0, jt, pt_hold[jt])
                    if pend is not None:
                        pv_mm(o_ps00, *pend)
                    pend = (jt, pt)
            pv_mm(o_ps00, *pend)
        drain_block(0, 0, o_ps00)

    # --- attention ---
    # (1,0) PV burst first: ACT still has the wave-3 exp backlog to chew
    o_ps10 = [
        opool.tile([DH + 1, IC], F32, name=f"o10_{i}", tag=f"ops{i}", bufs=1)
        for i in range(2)
    ]
    for jt in range(NJ):
        pv_mm(o_ps10, jt, pt_hold[jt])
    drain_block(1, 0, o_ps10)
    for hp in range(2):
        for ic in range(1, NI):
            o_ps = [
                opool.tile([DH + 1, IC], F32, name=f"ops{i}", tag=f"ops{i}", bufs=1)
                for i in range(2)
            ]
            pend = None
            for jt in range(NJ):
                pt = ptpool.tile([P, 2 * IC], F16, tag="pt")
                qk_exp(hp, ic, jt, pt)
                if pend is not None:
                    pv_mm(o_ps, *pend)
                pend = (jt, pt)
            pv_mm(o_ps, *pend)
            drain_block(hp, ic, o_ps)


_CACHE = {}


def build():
    if "nc" in _CACHE:
        return _CACHE["nc"]
    nc = bacc.Bacc(
        "TRN2", target_bir_lowering=False, debug=False, num_devices=NCORES
    )
    xT = nc.dram_tensor("xT", (DIM, N), F16, kind="ExternalInput").ap()
    wq = nc.dram_tensor("wq", (DIM, G * DH), F16, kind="ExternalInput").ap()
    wkv = nc.dram_tensor("wkv", (DIM, P), F16, kind="ExternalInput").ap()
    dupeye = nc.dram_tensor("dupeye", (DH, P), F16, kind="ExternalInput").ap()
    eyef = nc.dram_tensor("eyef", (DH, DH), F32, kind="ExternalInput").ap()
    oT = nc.dram_tensor("oT", (G * DH, N), F32, kind="ExternalOutput").ap()
    with tile.TileContext(nc) as tc:
        with ExitStack() as ctx:
            emit_kernel(ctx, tc, xT, wq, wkv, dupeye, eyef, oT)
    nc.compile()
    _CACHE["nc"] = nc
    return nc


def make_in_maps(x, Wq, Wkv, k_scale):
    x = np.asarray(x, dtype=np.float32)
    Wq = np.asarray(Wq, dtype=np.float32)
    Wkv = np.asarray(Wkv, dtype=np.float32)
    k_scale = np.asarray(k_scale, dtype=np.float32)
    xTs = [np.ascontiguousarray(x[b].T) for b in range(B)]
    eye = np.eye(DH, dtype=np.float16)
    dupeye = np.concatenate([eye, eye], axis=1)
    in_maps = []
    for c in range(NCORES):
        b, kv = divmod(c, KV_HEADS)
        # qk-norm: (q*ks)@k == q@(k*ks) — fold k_scale into Wq per q head
        wq_base = Wq[:, kv * G * DH : (kv + 1) * G * DH]
        wq_c = np.concatenate(
            [
                wq_base[:, j * DH : (j + 1) * DH] * k_scale[kv * G + j, 0][None, :]
                for j in range(G)
            ],
            axis=1,
        )
        wkv_c = np.concatenate(
            [
                Wkv[:, kv * DH : (kv + 1) * DH],
                Wkv[:, KV_HEADS * DH + kv * DH : KV_HEADS * DH + (kv + 1) * DH],
            ],
            axis=1,
        )
        in_maps.append(
            {
                "xT": xTs[b].astype(np.float16),
                "wq": np.ascontiguousarray(wq_c).astype(np.float16),
                "wkv": np.ascontiguousarray(wkv_c).astype(np.float16),
                "dupeye": dupeye,
                "eyef": np.eye(DH, dtype=np.float32),
            }
        )
    return in_maps


def gather(results):
    # normalization (divide by softmax sums) happens here on the host
    out = np.empty((B, N, HEADS * DH), dtype=np.float32)
    for c in range(NCORES):
        b, kv = divmod(c, KV_HEADS)
        oT = results[c]["oT"].reshape(G, DH, N) / results[c]["sums_d"][:, None, :]
        out[b, :, kv * G * DH : (kv + 1) * G * DH] = (
            oT.reshape(G * DH, N).T
        )
    return out


def kernel(x, Wq, Wkv, k_scale, _trace=False):
    nc = build()
    in_maps = make_in_maps(x, Wq, Wkv, k_scale)
    res = run_bass_kernel_spmd(
        nc, in_maps, core_ids=list(range(NCORES)), trace=_trace
    )
    out = gather(res.results)
    if _trace:
        kernel.last_result = res
    return out



# revision 45
# speedup vs baseline: 1.2264x; 1.2264x over previous
"""Trainium2 Bass kernel for grouped-query attention with qk-norm.

Problem (hardcoded): x(2,2048,1024) @ Wq(1024,1024) / Wkv(1024,512),
16 query heads, 4 kv heads, head_dim 64, k_scale(16,1,64) applied to the
group-broadcast k. Output (2,2048,1024).

Sharding: 8 cores = batch(2) x kv_heads(4). Each core computes its batch's
4 query heads against its kv head over the full 2048x2048 score matrix.

Device kernel layout choices:
- Host passes x transposed (xT: dim on partitions) so all projection
  matmuls contract over dim with no on-device transposes.
- k_scale is folded into Wk host-side: (x@Wk)*ks == x@(Wk*diag(ks)),
  giving a per-query-head scaled kkT directly from the projection.
- Scores are computed transposed (S^T: keys on partitions, queries free)
  so that exp(S^T) tiles feed the PV matmul directly as the moving
  operand (no P transpose).
- Softmax skips the max-subtraction (inputs are bounded; exp stays well
  inside fp32 range) and normalizes after PV via an appended ones-row in
  the V stationary operand (row 64 of the PV psum accumulates sum(exp)).
- Output is returned transposed per head (oT: 4*64 x 2048); the host
  transposes during the gather.
- Matmul inputs are float32r (fp32 storage, reduced-precision multiply,
  4x the fp32 throughput at moving-dim >= 256).
"""

import os
from contextlib import ExitStack

import numpy as np

import concourse.bacc as bacc
import concourse.mybir as mybir
import concourse.tile as tile
from concourse.bass_utils import run_bass_kernel_spmd

# Problem constants
B, N, DIM = 2, 2048, 1024
HEADS, KV_HEADS, DH = 16, 4, 64
G = HEADS // KV_HEADS  # query heads per kv head (4)
NCORES = 8
P = 128
KT = DIM // P  # 8 contraction tiles over dim
IC = 512  # query-chunk width
NI = N // IC  # 4
NJ = N // P  # 16 key tiles
SCALE = DH**-0.5

F32 = mybir.dt.float32
F16 = mybir.dt.float16
DMM = F16  # all matmul operands fp16: 1 row/cycle, fast stationary loads


def emit_kernel(ctx, tc, xT, wq, wkv, dupeye, eyef, oT):
    nc = tc.nc
    Exp = mybir.ActivationFunctionType.Exp
    mult = mybir.AluOpType.mult

    wpool = ctx.enter_context(tc.tile_pool(name="w", bufs=1))
    qkpool = ctx.enter_context(tc.tile_pool(name="qk", bufs=1))
    ptpool = ctx.enter_context(tc.tile_pool(name="pt", bufs=6))
    npool = ctx.enter_context(tc.tile_pool(name="norm", bufs=2))

    # --- persistent SBUF tensors ---
    ones_sb = wpool.tile([P, DH], DMM, tag="ones")  # warmup matmul operand
    # f32 identity at partitions 64-127 for the vT transpose (v lives there)
    eye_sb = wpool.tile([P, DH], F32, tag="eye")
    # [I | I] stationary that duplicates k rows 0-63 into psum rows 64-127
    dup_sb = wpool.tile([DH, P], DMM, tag="dup")
    qT = [qkpool.tile([P, N], DMM, name=f"qT{hp}", tag=f"qT{hp}") for hp in range(2)]
    # single unscaled k (k_scale folded into Wq host-side), duplicated on
    # partitions 0-63 and 64-127 to serve both QK tile positions
    kkT = qkpool.tile([P, N], DMM, tag="kkT")
    vaug = qkpool.tile([P, NJ * (DH + 1)], F16, tag="vaug")
    nc.any.memset(vaug[:], 1.0)
    nc.any.memset(ones_sb[:], 1.0)
    warm = qkpool.tile([1, 1], F32, tag="warm")
    nc.any.memset(warm[:], 1.0)
    nc.scalar.activation(warm[:], warm[:], Exp)
    nc.sync.dma_start(dup_sb[:], dupeye[:, :])
    nc.sync.dma_start(eye_sb[DH:P, :], eyef[:, :])

    # unnormalized output + softmax sums; the host divides during gather
    sums_d = nc.dram_tensor("sums_d", (G, N), F32, kind="ExternalOutput").ap()
    o_acc = [
        npool.tile([DH + 1, N], F32, name=f"oacc{h}", tag=f"oacc{h}", bufs=1)
        for h in range(G)
    ]

    def qk_exp(hp, ic, jt, pt):
        csl = slice(ic * IC, (ic + 1) * IC)
        st = apsum.tile([P, 2 * IC], F32, tag="s", bufs=3, name="st")
        for half in range(2):
            rsl = slice(half * 64, half * 64 + 64)
            nc.tensor.matmul(
                st[:, half * IC : (half + 1) * IC],
                kkT[rsl, jt * P : (jt + 1) * P],
                qT[hp][rsl, csl],
                start=True,
                stop=True,
                tile_position=(half * 64, 0),
            )
        nc.scalar.activation(pt[:], st[:], Exp, scale=SCALE)

    def pv_mm(o_ps, jt, pt):
        for half in range(2):
            nc.tensor.matmul(
                o_ps[half][:],
                vaug[:, jt * (DH + 1) : (jt + 1) * (DH + 1)],
                pt[:, half * IC : (half + 1) * IC],
                start=(jt == 0),
                stop=(jt == NJ - 1),
            )

    def attn_block(hp, ic, o_ps, jts):
        # issue QK(jt+1) before PV(jt): the PE queue is in-order, so PV
        # waiting on exp(jt) must not block the independent next QK
        pend = None
        for jt in jts:
            pt = ptpool.tile([P, 2 * IC], F16, tag="pt")
            qk_exp(hp, ic, jt, pt)
            if pend is not None:
                pv_mm(o_ps, *pend)
            pend = (jt, pt)
        pv_mm(o_ps, *pend)

    def drain_block(hp, ic, o_ps):
        for half in range(2):
            h = 2 * hp + half
            csl = slice(ic * IC, (ic + 1) * IC)
            nc.vector.tensor_copy(o_acc[h][:, csl], o_ps[half][:])
            nc.sync.dma_start(
                sums_d[h : h + 1, csl], o_acc[h][DH : DH + 1, csl]
            )
            nc.sync.dma_start(oT[h * DH : (h + 1) * DH, csl], o_acc[h][0:DH, csl])

    # S-tile pool lives for the whole kernel so early attention blocks can
    # overlap the projection phase (PV is deferred; its accumulator banks
    # open only after the projection psum pool closes).
    apsum = ctx.enter_context(tc.tile_pool(name="ap", bufs=3, space="PSUM"))
    # Dummy matmuls during the initial DMA wait keep the PE HAM activity
    # monitor busy so real projections start at 2.4GHz instead of 1.2.
    for _ in range(28):
        wt = apsum.tile([DH, IC], F32, tag="s", name="wt", bufs=3)
        nc.tensor.matmul(
            wt[:, 0:DH], ones_sb[:, 0:DH], ones_sb[:, 0:DH],
            start=True, stop=True,
        )

    # --- projections (fp16 inputs): qT (d on partitions) + packed [k|v] ---
    with tc.tile_pool(name="xw", bufs=1) as xwpool:
        wq_sb = xwpool.tile([P, KT * 256], F16, tag="wq")
        wkv_sb = xwpool.tile([P, KT * P], F16, tag="wkv")
        xts = xwpool.tile([P, KT * N], F16, tag="xt")  # 4MB
        # v occupies partitions 64-127 (its home in the packed kv psum)
        vT_sb = xwpool.tile([P, N], F32, tag="vT")

        qs = [nc.gpsimd, nc.scalar, nc.sync]

        def dma_x(kt, ic, eng):
            r = slice(kt * P, (kt + 1) * P)
            csl = slice(ic * IC, (ic + 1) * IC)
            eng.dma_start(
                xts[:, kt * N + ic * IC : kt * N + (ic + 1) * IC], xT[r, csl]
            )

        # weights first, spread over all three DMA queues, then x round-robin
        n = 0
        for kt in range(KT):
            r = slice(kt * P, (kt + 1) * P)
            qs[n % 3].dma_start(wq_sb[:, kt * 256 : (kt + 1) * 256], wq[r, :])
            qs[(n + 1) % 3].dma_start(wkv_sb[:, kt * P : (kt + 1) * P], wkv[r, :])
            n += 2
        for ic in range(NI):
            for kt in range(KT):
                dma_x(kt, ic, qs[n % 3])
                n += 1

        def proj_wave(ic, pp):
            # one wave = every projection chain that consumes xts chunk ic
            csl = slice(ic * IC, (ic + 1) * IC)
            for hp in range(2):
                ps = pp.tile([P, IC], F32, tag="pj", name="pjt", bufs=2)
                for kt in range(KT):
                    c0 = kt * 256 + hp * 128
                    nc.tensor.matmul(
                        ps[:],
                        wq_sb[:, c0 : c0 + 128],
                        xts[:, kt * N + ic * IC : kt * N + (ic + 1) * IC],
                        start=(kt == 0),
                        stop=(kt == KT - 1),
                    )
                nc.vector.tensor_copy(qT[hp][:, csl], ps[:])
            # packed [k | v] projection: k on rows 0-63, v on rows 64-127
            ps = pp.tile([P, IC], F32, tag="pj", name="pjkv", bufs=2)
            for kt in range(KT):
                nc.tensor.matmul(
                    ps[:],
                    wkv_sb[:, kt * P : (kt + 1) * P],
                    xts[:, kt * N + ic * IC : kt * N + (ic + 1) * IC],
                    start=(kt == 0),
                    stop=(kt == KT - 1),
                )
            nc.vector.tensor_copy(kkT[0:DH, csl], ps[0:DH, :])
            nc.vector.tensor_copy(vT_sb[DH:P, csl], ps[DH:P, :])
            # duplicate k into kkT rows 64-127 via the [I|I] stationary
            psd = pp.tile([P, IC], F32, tag="pj", name="pjd", bufs=2)
            nc.tensor.matmul(
                psd[:], dup_sb[:], kkT[0:DH, csl], start=True, stop=True
            )
            nc.vector.tensor_copy(kkT[DH:P, csl], psd[DH:P, :])
            for jt in range(4 * ic, 4 * ic + 4):
                pv = pp.tile([P, DH], F32, tag="pj", bufs=2, name="pvt")
                nc.tensor.transpose(
                    pv[:], vT_sb[DH:P, jt * P : (jt + 1) * P], eye_sb[DH:P, :]
                )
                nc.vector.tensor_copy(
                    vaug[:, jt * (DH + 1) : jt * (DH + 1) + DH], pv[:]
                )

        pt_hold = [
            ptpool.tile([P, 2 * IC], F16, name=f"pth{j}", tag=f"pth{j}", bufs=1)
            for j in range(NJ)
        ]
        with tc.tile_pool(name="pp", bufs=2, space="PSUM") as pp:
            # early QK+exp for (hp0, ic0) follows each wave as its keys land,
            # keeping the ACT engine fed through the whole projection phase;
            # PV runs later (accumulator banks open after this pool closes).
            proj_wave(0, pp)
            for jt in range(0, 4):
                qk_exp(0, 0, jt, pt_hold[jt])
            proj_wave(1, pp)
            for jt in range(4, 8):
                qk_exp(0, 0, jt, pt_hold[jt])
            proj_wave(2, pp)
            for jt in range(8, 12):
                qk_exp(0, 0, jt, pt_hold[jt])
            proj_wave(3, pp)
            for jt in range(12, 16):
                qk_exp(0, 0, jt, pt_hold[jt])

    # --- attention ---
    with tc.tile_pool(name="op", bufs=1, space="PSUM") as opool:
        for hp in range(2):
            for ic in range(NI):
                o_ps = [
                    opool.tile(
                        [DH + 1, IC], F32, name=f"ops{i}", tag=f"ops{i}", bufs=1
                    )
                    for i in range(2)
                ]
                if hp == 0 and ic == 0:
                    for jt in range(NJ):
                        pv_mm(o_ps, jt, pt_hold[jt])
                else:
                    attn_block(hp, ic, o_ps, range(NJ))
                drain_block(hp, ic, o_ps)


_CACHE = {}


def build():
    if "nc" in _CACHE:
        return _CACHE["nc"]
    nc = bacc.Bacc(
        "TRN2", target_bir_lowering=False, debug=False, num_devices=NCORES
    )
    xT = nc.dram_tensor("xT", (DIM, N), F16, kind="ExternalInput").ap()
    wq = nc.dram_tensor("wq", (DIM, G * DH), F16, kind="ExternalInput").ap()
    wkv = nc.dram_tensor("wkv", (DIM, P), F16, kind="ExternalInput").ap()
    dupeye = nc.dram_tensor("dupeye", (DH, P), F16, kind="ExternalInput").ap()
    eyef = nc.dram_tensor("eyef", (DH, DH), F32, kind="ExternalInput").ap()
    oT = nc.dram_tensor("oT", (G * DH, N), F32, kind="ExternalOutput").ap()
    with tile.TileContext(nc) as tc:
        with ExitStack() as ctx:
            emit_kernel(ctx, tc, xT, wq, wkv, dupeye, eyef, oT)
    nc.compile()
    _CACHE["nc"] = nc
    return nc


def make_in_maps(x, Wq, Wkv, k_scale):
    x = np.asarray(x, dtype=np.float32)
    Wq = np.asarray(Wq, dtype=np.float32)
    Wkv = np.asarray(Wkv, dtype=np.float32)
    k_scale = np.asarray(k_scale, dtype=np.float32)
    xTs = [np.ascontiguousarray(x[b].T) for b in range(B)]
    eye = np.eye(DH, dtype=np.float16)
    dupeye = np.concatenate([eye, eye], axis=1)
    in_maps = []
    for c in range(NCORES):
        b, kv = divmod(c, KV_HEADS)
        # qk-norm: (q*ks)@k == q@(k*ks) — fold k_scale into Wq per q head
        wq_base = Wq[:, kv * G * DH : (kv + 1) * G * DH]
        wq_c = np.concatenate(
            [
                wq_base[:, j * DH : (j + 1) * DH] * k_scale[kv * G + j, 0][None, :]
                for j in range(G)
            ],
            axis=1,
        )
        wkv_c = np.concatenate(
            [
                Wkv[:, kv * DH : (kv + 1) * DH],
                Wkv[:, KV_HEADS * DH + kv * DH : KV_HEADS * DH + (kv + 1) * DH],
            ],
            axis=1,
        )
        in_maps.append(
            {
                "xT": xTs[b].astype(np.float16),
                "wq": np.ascontiguousarray(wq_c).astype(np.float16),
                "wkv": np.ascontiguousarray(wkv_c).astype(np.float16),
                "dupeye": dupeye,
                "eyef": np.eye(DH, dtype=np.float32),
            }
        )
    return in_maps


def gather(results):
    # normalization (divide by softmax sums) happens here on the host
    out = np.empty((B, N, HEADS * DH), dtype=np.float32)
    for c in range(NCORES):
        b, kv = divmod(c, KV_HEADS)
        oT = results[c]["oT"].reshape(G, DH, N) / results[c]["sums_d"][:, None, :]
        out[b, :, kv * G * DH : (kv + 1) * G * DH] = (
            oT.reshape(G * DH, N).T
        )
    return out


def kernel(x, Wq, Wkv, k_scale, _trace=False):
    nc = build()
    in_maps = make_in_maps(x, Wq, Wkv, k_scale)
    res = run_bass_kernel_spmd(
        nc, in_maps, core_ids=list(range(NCORES)), trace=_trace
    )
    out = gather(res.results)
    if _trace:
        kernel.last_result = res
    return out



# revision 46
# speedup vs baseline: 1.2569x; 1.0249x over previous
"""Trainium2 Bass kernel for grouped-query attention with qk-norm.

Problem (hardcoded): x(2,2048,1024) @ Wq(1024,1024) / Wkv(1024,512),
16 query heads, 4 kv heads, head_dim 64, k_scale(16,1,64) applied to the
group-broadcast k. Output (2,2048,1024).

Sharding: 8 cores = batch(2) x kv_heads(4). Each core computes its batch's
4 query heads against its kv head over the full 2048x2048 score matrix.

Device kernel layout choices:
- Host passes x transposed (xT: dim on partitions) so all projection
  matmuls contract over dim with no on-device transposes.
- k_scale is folded into Wk host-side: (x@Wk)*ks == x@(Wk*diag(ks)),
  giving a per-query-head scaled kkT directly from the projection.
- Scores are computed transposed (S^T: keys on partitions, queries free)
  so that exp(S^T) tiles feed the PV matmul directly as the moving
  operand (no P transpose).
- Softmax skips the max-subtraction (inputs are bounded; exp stays well
  inside fp32 range) and normalizes after PV via an appended ones-row in
  the V stationary operand (row 64 of the PV psum accumulates sum(exp)).
- Output is returned transposed per head (oT: 4*64 x 2048); the host
  transposes during the gather.
- Matmul inputs are float32r (fp32 storage, reduced-precision multiply,
  4x the fp32 throughput at moving-dim >= 256).
"""

import os
from contextlib import ExitStack

import numpy as np

import concourse.bacc as bacc
import concourse.mybir as mybir
import concourse.tile as tile
from concourse.bass_utils import run_bass_kernel_spmd

# Problem constants
B, N, DIM = 2, 2048, 1024
HEADS, KV_HEADS, DH = 16, 4, 64
G = HEADS // KV_HEADS  # query heads per kv head (4)
NCORES = 8
P = 128
KT = DIM // P  # 8 contraction tiles over dim
IC = 512  # query-chunk width
NI = N // IC  # 4
NJ = N // P  # 16 key tiles
SCALE = DH**-0.5

F32 = mybir.dt.float32
F16 = mybir.dt.float16
DMM = F16  # all matmul operands fp16: 1 row/cycle, fast stationary loads


def emit_kernel(ctx, tc, xT, wq, wkv, dupeye, eyef, oT):
    nc = tc.nc
    Exp = mybir.ActivationFunctionType.Exp
    mult = mybir.AluOpType.mult

    wpool = ctx.enter_context(tc.tile_pool(name="w", bufs=1))
    qkpool = ctx.enter_context(tc.tile_pool(name="qk", bufs=1))
    ptpool = ctx.enter_context(tc.tile_pool(name="pt", bufs=6))
    npool = ctx.enter_context(tc.tile_pool(name="norm", bufs=2))

    # --- persistent SBUF tensors ---
    ones_sb = wpool.tile([P, DH], DMM, tag="ones")  # warmup matmul operand
    # f32 identity at partitions 64-127 for the vT transpose (v lives there)
    eye_sb = wpool.tile([P, DH], F32, tag="eye")
    # [I | I] stationary that duplicates k rows 0-63 into psum rows 64-127
    dup_sb = wpool.tile([DH, P], DMM, tag="dup")
    qT = [qkpool.tile([P, N], DMM, name=f"qT{hp}", tag=f"qT{hp}") for hp in range(2)]
    # single unscaled k (k_scale folded into Wq host-side), duplicated on
    # partitions 0-63 and 64-127 to serve both QK tile positions
    kkT = qkpool.tile([P, N], DMM, tag="kkT")
    vaug = qkpool.tile([P, NJ * (DH + 1)], F16, tag="vaug")
    nc.any.memset(vaug[:], 1.0)
    nc.any.memset(ones_sb[:], 1.0)
    warm = qkpool.tile([1, 1], F32, tag="warm")
    nc.any.memset(warm[:], 1.0)
    nc.scalar.activation(warm[:], warm[:], Exp)
    nc.sync.dma_start(dup_sb[:], dupeye[:, :])
    nc.sync.dma_start(eye_sb[DH:P, :], eyef[:, :])

    # unnormalized output + softmax sums; the host divides during gather
    sums_d = nc.dram_tensor("sums_d", (G, N), F32, kind="ExternalOutput").ap()
    o_acc = [
        npool.tile([DH + 1, N], F32, name=f"oacc{h}", tag=f"oacc{h}", bufs=1)
        for h in range(G)
    ]

    def qk_exp(hp, ic, jt, pt):
        csl = slice(ic * IC, (ic + 1) * IC)
        st = apsum.tile([P, 2 * IC], F32, tag="s", bufs=3, name="st")
        for half in range(2):
            rsl = slice(half * 64, half * 64 + 64)
            nc.tensor.matmul(
                st[:, half * IC : (half + 1) * IC],
                kkT[rsl, jt * P : (jt + 1) * P],
                qT[hp][rsl, csl],
                start=True,
                stop=True,
                tile_position=(half * 64, 0),
            )
        nc.scalar.activation(pt[:], st[:], Exp, scale=SCALE)

    def pv_mm(o_ps, jt, pt):
        for half in range(2):
            nc.tensor.matmul(
                o_ps[half][:],
                vaug[:, jt * (DH + 1) : (jt + 1) * (DH + 1)],
                pt[:, half * IC : (half + 1) * IC],
                start=(jt == 0),
                stop=(jt == NJ - 1),
            )

    def attn_block(hp, ic, o_ps, jts):
        # issue QK(jt+1) before PV(jt): the PE queue is in-order, so PV
        # waiting on exp(jt) must not block the independent next QK
        pend = None
        for jt in jts:
            pt = ptpool.tile([P, 2 * IC], F16, tag="pt")
            qk_exp(hp, ic, jt, pt)
            if pend is not None:
                pv_mm(o_ps, *pend)
            pend = (jt, pt)
        pv_mm(o_ps, *pend)

    def drain_block(hp, ic, o_ps):
        for half in range(2):
            h = 2 * hp + half
            csl = slice(ic * IC, (ic + 1) * IC)
            nc.vector.tensor_copy(o_acc[h][:, csl], o_ps[half][:])
            nc.sync.dma_start(
                sums_d[h : h + 1, csl], o_acc[h][DH : DH + 1, csl]
            )
            nc.sync.dma_start(oT[h * DH : (h + 1) * DH, csl], o_acc[h][0:DH, csl])

    # S-tile pool lives for the whole kernel so early attention blocks can
    # overlap the projection phase (PV is deferred; its accumulator banks
    # open only after the projection psum pool closes).
    apsum = ctx.enter_context(tc.tile_pool(name="ap", bufs=3, space="PSUM"))
    # Dummy matmuls during the initial DMA wait keep the PE HAM activity
    # monitor busy so real projections start at 2.4GHz instead of 1.2.
    for _ in range(28):
        wt = apsum.tile([DH, IC], F32, tag="s", name="wt", bufs=3)
        nc.tensor.matmul(
            wt[:, 0:DH], ones_sb[:, 0:DH], ones_sb[:, 0:DH],
            start=True, stop=True,
        )

    # --- projections (fp16 inputs): qT (d on partitions) + packed [k|v] ---
    with tc.tile_pool(name="xw", bufs=1) as xwpool:
        wq_sb = xwpool.tile([P, KT * 256], F16, tag="wq")
        wkv_sb = xwpool.tile([P, KT * P], F16, tag="wkv")
        xts = xwpool.tile([P, KT * N], F16, tag="xt")  # 4MB
        # v occupies partitions 64-127 (its home in the packed kv psum)
        vT_sb = xwpool.tile([P, N], F32, tag="vT")

        qs = [nc.gpsimd, nc.scalar, nc.sync]

        def dma_x(kt, ic, eng):
            r = slice(kt * P, (kt + 1) * P)
            csl = slice(ic * IC, (ic + 1) * IC)
            eng.dma_start(
                xts[:, kt * N + ic * IC : kt * N + (ic + 1) * IC], xT[r, csl]
            )

        # weights first, spread over all three DMA queues, then x round-robin
        n = 0
        for kt in range(KT):
            r = slice(kt * P, (kt + 1) * P)
            qs[n % 3].dma_start(wq_sb[:, kt * 256 : (kt + 1) * 256], wq[r, :])
            qs[(n + 1) % 3].dma_start(wkv_sb[:, kt * P : (kt + 1) * P], wkv[r, :])
            n += 2
        for ic in range(NI):
            for kt in range(KT):
                dma_x(kt, ic, qs[n % 3])
                n += 1

        def proj_wave(ic, pp):
            # one wave = every projection chain that consumes xts chunk ic
            csl = slice(ic * IC, (ic + 1) * IC)
            for hp in range(2):
                ps = pp.tile([P, IC], F32, tag="pj", name="pjt", bufs=2)
                for kt in range(KT):
                    c0 = kt * 256 + hp * 128
                    nc.tensor.matmul(
                        ps[:],
                        wq_sb[:, c0 : c0 + 128],
                        xts[:, kt * N + ic * IC : kt * N + (ic + 1) * IC],
                        start=(kt == 0),
                        stop=(kt == KT - 1),
                    )
                nc.vector.tensor_copy(qT[hp][:, csl], ps[:])
            # packed [k | v] projection: k on rows 0-63, v on rows 64-127
            ps = pp.tile([P, IC], F32, tag="pj", name="pjkv", bufs=2)
            for kt in range(KT):
                nc.tensor.matmul(
                    ps[:],
                    wkv_sb[:, kt * P : (kt + 1) * P],
                    xts[:, kt * N + ic * IC : kt * N + (ic + 1) * IC],
                    start=(kt == 0),
                    stop=(kt == KT - 1),
                )
            nc.vector.tensor_copy(kkT[0:DH, csl], ps[0:DH, :])
            nc.vector.tensor_copy(vT_sb[DH:P, csl], ps[DH:P, :])
            # duplicate k into kkT rows 64-127 via the [I|I] stationary
            psd = pp.tile([P, IC], F32, tag="pj", name="pjd", bufs=2)
            nc.tensor.matmul(
                psd[:], dup_sb[:], kkT[0:DH, csl], start=True, stop=True
            )
            nc.vector.tensor_copy(kkT[DH:P, csl], psd[DH:P, :])
            for jt in range(4 * ic, 4 * ic + 4):
                pv = pp.tile([P, DH], F32, tag="pj", bufs=2, name="pvt")
                nc.tensor.transpose(
                    pv[:], vT_sb[DH:P, jt * P : (jt + 1) * P], eye_sb[DH:P, :]
                )
                nc.vector.tensor_copy(
                    vaug[:, jt * (DH + 1) : jt * (DH + 1) + DH], pv[:]
                )

        # hold QK+exp for BOTH ic=0 blocks (hp 0 and 1) through the waves:
        # ACT stays fed the whole projection phase and builds a backlog that
        # covers the deferred PV bursts at attention start.
        pt_hold = [
            [
                ptpool.tile(
                    [P, 2 * IC], F16, name=f"pth{hp}_{j}", tag=f"pth{hp}_{j}", bufs=1
                )
                for j in range(NJ)
            ]
            for hp in range(2)
        ]
        with tc.tile_pool(name="pp", bufs=2, space="PSUM") as pp:
            for w in range(NI):
                proj_wave(w, pp)
                for jt in range(4 * w, 4 * w + 4):
                    qk_exp(0, 0, jt, pt_hold[0][jt])
                    qk_exp(1, 0, jt, pt_hold[1][jt])

    # --- attention ---
    with tc.tile_pool(name="op", bufs=1, space="PSUM") as opool:
        for hp in range(2):
            o_ps = [
                opool.tile(
                    [DH + 1, IC], F32, name=f"ops{i}", tag=f"ops{i}", bufs=1
                )
                for i in range(2)
            ]
            for jt in range(NJ):
                pv_mm(o_ps, jt, pt_hold[hp][jt])
            drain_block(hp, 0, o_ps)
        for hp in range(2):
            for ic in range(1, NI):
                o_ps = [
                    opool.tile(
                        [DH + 1, IC], F32, name=f"ops{i}", tag=f"ops{i}", bufs=1
                    )
                    for i in range(2)
                ]
                attn_block(hp, ic, o_ps, range(NJ))
                drain_block(hp, ic, o_ps)


_CACHE = {}


def build():
    if "nc" in _CACHE:
        return _CACHE["nc"]
    nc = bacc.Bacc(
        "TRN2", target_bir_lowering=False, debug=False, num_devices=NCORES
    )
    xT = nc.dram_tensor("xT", (DIM, N), F16, kind="ExternalInput").ap()
    wq = nc.dram_tensor("wq", (DIM, G * DH), F16, kind="ExternalInput").ap()
    wkv = nc.dram_tensor("wkv", (DIM, P), F16, kind="ExternalInput").ap()
    dupeye = nc.dram_tensor("dupeye", (DH, P), F16, kind="ExternalInput").ap()
    eyef = nc.dram_tensor("eyef", (DH, DH), F32, kind="ExternalInput").ap()
    oT = nc.dram_tensor("oT", (G * DH, N), F32, kind="ExternalOutput").ap()
    with tile.TileContext(nc) as tc:
        with ExitStack() as ctx:
            emit_kernel(ctx, tc, xT, wq, wkv, dupeye, eyef, oT)
    nc.compile()
    _CACHE["nc"] = nc
    return nc


def make_in_maps(x, Wq, Wkv, k_scale):
    x = np.asarray(x, dtype=np.float32)
    Wq = np.asarray(Wq, dtype=np.float32)
    Wkv = np.asarray(Wkv, dtype=np.float32)
    k_scale = np.asarray(k_scale, dtype=np.float32)
    xTs = [np.ascontiguousarray(x[b].T) for b in range(B)]
    eye = np.eye(DH, dtype=np.float16)
    dupeye = np.concatenate([eye, eye], axis=1)
    in_maps = []
    for c in range(NCORES):
        b, kv = divmod(c, KV_HEADS)
        # qk-norm: (q*ks)@k == q@(k*ks) — fold k_scale into Wq per q head
        wq_base = Wq[:, kv * G * DH : (kv + 1) * G * DH]
        wq_c = np.concatenate(
            [
                wq_base[:, j * DH : (j + 1) * DH] * k_scale[kv * G + j, 0][None, :]
                for j in range(G)
            ],
            axis=1,
        )
        wkv_c = np.concatenate(
            [
                Wkv[:, kv * DH : (kv + 1) * DH],
                Wkv[:, KV_HEADS * DH + kv * DH : KV_HEADS * DH + (kv + 1) * DH],
            ],
            axis=1,
        )
        in_maps.append(
            {
                "xT": xTs[b].astype(np.float16),
                "wq": np.ascontiguousarray(wq_c).astype(np.float16),
                "wkv": np.ascontiguousarray(wkv_c).astype(np.float16),
                "dupeye": dupeye,
                "eyef": np.eye(DH, dtype=np.float32),
            }
        )
    return in_maps


def gather(results):
    # normalization (divide by softmax sums) happens here on the host
    out = np.empty((B, N, HEADS * DH), dtype=np.float32)
    for c in range(NCORES):
        b, kv = divmod(c, KV_HEADS)
        oT = results[c]["oT"].reshape(G, DH, N) / results[c]["sums_d"][:, None, :]
        out[b, :, kv * G * DH : (kv + 1) * G * DH] = (
            oT.reshape(G * DH, N).T
        )
    return out


def kernel(x, Wq, Wkv, k_scale, _trace=False):
    nc = build()
    in_maps = make_in_maps(x, Wq, Wkv, k_scale)
    res = run_bass_kernel_spmd(
        nc, in_maps, core_ids=list(range(NCORES)), trace=_trace
    )
    out = gather(res.results)
    if _trace:
        kernel.last_result = res
    return out

